# revision 28
# baseline (speedup 1.0000x reference)
"""Multi-head attention (B=2, S=2048, E=1024, H=16, D=64) on 8 TRN2 NeuronCores.

Sharding: data parallel over batch (2) x head-group parallel (4 groups of 4
heads). Each core computes Q/K/V projections for its 4 heads, full-sequence
attention for those heads, and a partial output projection (its heads' rows of
Wo). The host sums the 4 partial outputs per batch and adds the bias.

On-device layout (all matmul operands bf16, accumulation fp32):
  - x is fed pre-transposed per batch: xT [E, S]. Input DMA is ordered
    first-needed-first: x in 512-wide s-slices in the (ec-pair, s-half) order
    the K0 prologue consumes them, round-robin across three DMA queues;
    weights go chunk-granular on the scalar queue so the first matmul waits
    only for its own wk pair + 4 x chunks (~4us) instead of the full loads.
  - Q^T, K^T are computed head-transposed [dh, S] so that attention logits are
    produced key-partitioned: logitsT[k, q] = sum_d K^T[d,k] Q^T[d,q]. Softmax
    (no max subtraction -- logits are provably tiny) is exp on ScalarE plus a
    partition-reduction done with an all-ones matmul; attn@V then consumes the
    exp tiles directly as the moving operand with V [s, dh] as stationary.
  - attn@V and the ones-reduction accumulate both 64-row key-subchunks into
    the SAME psum region (the psum accumulate port makes concurrent quadrant
    drains safe), so psO/psS are one bank each: psO double-buffers (removing
    the per-iteration WAR stall) and the tail is rec+mul straight out of PSUM.
  - Projections and the output projection run the PE in plain 128x128 mode;
    the attention core (logits / attn@V / ones-reduction) runs in 64x64 array
    tiling so the d=64-contraction matmuls use the full array.
  - Q/K projections run ec-outer/i-inner so consecutive matmuls share their
    stationary weight chunk; the second K block drains via the urgent queue
    inside the first attention slots instead of serializing the prologue.
  - PSUM evacuations for V and the output projection run on GpSimd (idle
    otherwise); VectorE keeps the exp-tree adds and the softmax tail so the
    ScalarE exp cadence and the PE stay the pacers. Output is stored bf16 and
    summed on host.
"""

import os
import numpy as np
import ml_dtypes

import concourse.bass as bass
import concourse.mybir as mybir
import concourse.tile as tile
from concourse import bacc
from concourse import bass_utils
from contextlib import ExitStack

# bisect flags (default = all optimizations on)
# 0 = baseline DMA scheme. The restructured schemes (1/2) measured neutral
# (prologue is bounded by the ~8.7us DMA-ring spin-up, not trigger order),
# and scheme 2 correlated with a chip clock-state that runs all non-PE
# engines at 1.0GHz instead of 1.2GHz (+19% total time).
DMA_V2 = int(os.environ.get("K_DMA_V2", "0"))
# same-region concurrent-quadrant PSUM accumulation fails at runtime on hw
# (works in CoreSim) -- keep disjoint g-regions + evac add.
G_ACCUM = os.environ.get("K_G_ACCUM", "0") == "1"
# number of the 8 per-iteration s1 tree-adds to run on GpSimd (2.1us each
# there vs 0.4us on DVE -- GpSimd saturates above ~4)
S1_GP_N = int(os.environ.get("K_S1_GP_N", "0"))
# kc slots whose exp runs as a one-op Schraudolph approximation on the DVE
# (int16 bitcast trick) instead of ScalarE: relieves the exp cadence.
# Max rel err of the approx ~3.3%; softmax normalization cancels most of it
# (measured end-to-end: 3 slots -> 0.0061 vs tolerance 2e-2).
SCHRAU_KC = frozenset(
    int(c) for c in os.environ.get("K_SCHRAU_KC", "4,6").split(",") if c != "")
S16 = 128.0 / float(np.log(2.0))
SCHRAU_BIAS = 16256.0 - 5.0  # truncation-calibrated sigma
# double-buffer psO (psum shared pool drops to 2 bufs to fit 8 banks)
PSO2 = os.environ.get("K_PSO2", "1") == "1"

P = 128
B, S, E = 2, 2048, 1024
H, D = 16, 64
NCORES = 8
GROUPS = NCORES // B          # 4 head-groups per batch
HPG = H // GROUPS             # 4 heads per core
DHG = HPG * D                 # 256 head dims per core
NHP = HPG // 2                # 2 head-pairs per core
EC = E // P                   # 8 e-chunks of 128
KC = S // P                   # 16 key chunks of 128
QCW = 512                     # q-chunk width
NQC = S // QCW                # 4 q chunks
SCALE = float(D) ** -0.5

BF16 = mybir.dt.bfloat16
F32 = mybir.dt.float32
EXP = mybir.ActivationFunctionType.Exp

_NC = None


def _emit(tc):
    nc = tc.nc
    xT = nc.dram_tensor("xT", [E, S], BF16, kind="ExternalInput").ap()
    wqT = nc.dram_tensor("wqT", [E, DHG], BF16, kind="ExternalInput").ap()
    wkT = nc.dram_tensor("wkT", [E, DHG], BF16, kind="ExternalInput").ap()
    wvT = nc.dram_tensor("wvT", [E, DHG], BF16, kind="ExternalInput").ap()
    woT = nc.dram_tensor("woT", [DHG, E], BF16, kind="ExternalInput").ap()
    outT = nc.dram_tensor("outT", [E, S], BF16, kind="ExternalOutput").ap()

    mm = nc.tensor.matmul

    with ExitStack() as ctx:
        consts = ctx.enter_context(tc.tile_pool(name="consts", bufs=1))
        xp = ctx.enter_context(tc.tile_pool(name="xp", bufs=1))
        qkvp = ctx.enter_context(tc.tile_pool(name="qkvp", bufs=1))
        psum = ctx.enter_context(tc.tile_pool(
            name="psum", bufs=2 if PSO2 else 3, space="PSUM"))
        psum_o = ctx.enter_context(tc.tile_pool(
            name="psum_o", bufs=2 if (G_ACCUM or PSO2) else 1, space="PSUM"))
        expp = ctx.enter_context(tc.tile_pool(name="expp", bufs=2))
        s1p = ctx.enter_context(tc.tile_pool(name="s1p", bufs=1))
        smallp = ctx.enter_context(tc.tile_pool(name="smallp", bufs=2))
        otnp = ctx.enter_context(tc.tile_pool(name="otnp", bufs=6))
        outfp = ctx.enter_context(tc.tile_pool(name="outfp", bufs=3))

        # ---- input loads: x first-needed-first on 3 queues, weights
        # chunk-granular on the (initially idle) scalar queue so the first
        # matmul waits only for wk pair 0 and the first 4 x chunks.
        wq_sb = consts.tile([P, EC, DHG], BF16)
        wk_sb = consts.tile([P, EC, DHG], BF16)
        wv_sb = consts.tile([P, EC, DHG], BF16)
        wo_sb = consts.tile([P, NHP, E], BF16)
        ones = consts.tile([P, 64], BF16)
        nc.vector.memset(ones, 1.0)
        x_sb = xp.tile([P, EC, S], BF16)
        dma_eng = [nc.sync, nc.scalar, nc.gpsimd]
        if DMA_V2 == 2:
            # chunked weights on scalar, ec-pair-merged x on sync+gpsimd
            wkr = wkT.rearrange("(c p) d -> p c d", p=P)
            wqr = wqT.rearrange("(c p) d -> p c d", p=P)
            xr = xT.rearrange("(c p) q -> p c q", p=P)
            for e2 in range(4):
                nc.scalar.dma_start(wk_sb[:, 2 * e2:2 * e2 + 2, :],
                                    wkr[:, 2 * e2:2 * e2 + 2, :])
            for e2 in range(4):
                nc.scalar.dma_start(wq_sb[:, 2 * e2:2 * e2 + 2, :],
                                    wqr[:, 2 * e2:2 * e2 + 2, :])
            nc.scalar.dma_start(wv_sb, wvT.rearrange("(c p) d -> p c d", p=P))
            nc.scalar.dma_start(wo_sb, woT.rearrange("(h p) e -> p h e", p=P))
            order = [(e2, sq) for e2 in range(4) for sq in (0, 1)]
            order += [(e2, sq) for sq in (2, 3) for e2 in range(4)]
            for n, (e2, sq) in enumerate(order):
                ssl = slice(sq * QCW, (sq + 1) * QCW)
                (nc.sync if n % 2 == 0 else nc.gpsimd).dma_start(
                    x_sb[:, 2 * e2:2 * e2 + 2, ssl],
                    xr[:, 2 * e2:2 * e2 + 2, ssl])
        elif DMA_V2 == 1:
            # baseline scheme + only wk split in half and x ec-pair order,
            # to pull the first matmul earlier without changing queue shape
            wkr = wkT.rearrange("(c p) d -> p c d", p=P)
            nc.scalar.dma_start(wk_sb[:, 0:2, :], wkr[:, 0:2, :])
            nc.scalar.dma_start(wk_sb[:, 2:EC, :], wkr[:, 2:EC, :])
            order = [(ec, sq) for e2 in range(4) for ec in (2 * e2, 2 * e2 + 1)
                     for sq in (0, 1)]
            order += [(ec, sq) for sq in (2, 3) for ec in range(EC)]
            for n, (ec, sq) in enumerate(order):
                ssl = slice(sq * QCW, (sq + 1) * QCW)
                dma_eng[n % 3].dma_start(
                    x_sb[:, ec, ssl], xT[ec * P:(ec + 1) * P, ssl])
                if n == 15:
                    nc.scalar.dma_start(
                        wq_sb, wqT.rearrange("(c p) d -> p c d", p=P))
                if n == 23:
                    nc.scalar.dma_start(
                        wv_sb, wvT.rearrange("(c p) d -> p c d", p=P))
            nc.scalar.dma_start(wo_sb, woT.rearrange("(h p) e -> p h e", p=P))
        else:
            # baseline scheme; DMA_V2==3 only pulls the wq/wv triggers to
            # earlier positions on the scalar queue (same queue shapes).
            wq_pos, wv_pos = (6, 12) if DMA_V2 == 3 else (15, 23)
            nc.scalar.dma_start(wk_sb, wkT.rearrange("(c p) d -> p c d", p=P))
            order = [(ec, sq) for ec in range(EC) for sq in (0, 1)]
            order += [(ec, sq) for sq in (2, 3) for ec in range(EC)]
            for n, (ec, sq) in enumerate(order):
                ssl = slice(sq * QCW, (sq + 1) * QCW)
                dma_eng[n % 3].dma_start(
                    x_sb[:, ec, ssl], xT[ec * P:(ec + 1) * P, ssl])
                if n == wq_pos:
                    nc.scalar.dma_start(
                        wq_sb, wqT.rearrange("(c p) d -> p c d", p=P))
                if n == wv_pos:
                    nc.scalar.dma_start(
                        wv_sb, wvT.rearrange("(c p) d -> p c d", p=P))
            nc.scalar.dma_start(wo_sb, woT.rearrange("(h p) e -> p h e", p=P))

        qt_sb = qkvp.tile([P, NHP, S], BF16)   # [dh-pair, hp, s]
        kt_sb = qkvp.tile([P, NHP, S], BF16)
        v_sb = qkvp.tile([P, KC, DHG], BF16)   # [s%128, s-chunk, dh]

        # deferred-work queue: (approx_pe_ns, closure) drained into kc slots
        eager = []  # V chunks: exactly one per slot, always ahead of attnV
        pend = []   # urgent: per-iteration tails (free pools quickly)
        lazy = []   # deferred projections + output projection
        acc = [0.0, 0.0]

        def drain(b_urgent, b_lazy):
            acc[0] += b_urgent
            while pend and pend[0][0] <= acc[0]:
                cost, fn = pend.pop(0)
                fn()
                acc[0] -= cost
            if not pend:
                acc[0] = 0.0
            if eager:
                eager.pop(0)()
                return
            acc[1] += b_lazy
            while lazy and lazy[0][0] <= acc[1]:
                cost, fn = lazy.pop(0)
                fn()
                acc[1] -= cost
            if not lazy:
                acc[1] = 0.0

        # ---- phase 1 emitters (plain 128x128 array mode) ----
        def v_half(sc2, i, box):
            if i == 0:
                box.append(psum.tile([P, 2, QCW], F32, tag="ps", name="psV"))
            ps = box[0]
            sc = 2 * sc2 + i
            for ec in range(EC):
                mm(ps[:, i, 0:DHG],
                   lhsT=x_sb[:, ec, sc * P:(sc + 1) * P],
                   rhs=wv_sb[:, ec, :],
                   start=(ec == 0), stop=(ec == EC - 1),
                   skip_group_check=True)
            nc.vector.tensor_copy(v_sb[:, sc:sc + 1, :], ps[:, i:i + 1, 0:DHG])

        def qk_quarter(hp, w_sb, dst, sc2, e2, box):
            # ec-outer / i-inner: the two 512-q halves share each weight
            # chunk, so consecutive matmuls keep the same stationary.
            if e2 == 0:
                box.append(psum.tile([P, 2, QCW], F32, tag="ps", name="psQK"))
            ps = box[0]
            for ec in range(e2 * 2, e2 * 2 + 2):
                for i in range(2):
                    ssl = slice((2 * sc2 + i) * QCW, (2 * sc2 + i + 1) * QCW)
                    mm(ps[:, i, :],
                       lhsT=w_sb[:, ec, hp * P:(hp + 1) * P],
                       rhs=x_sb[:, ec, ssl],
                       start=(ec == 0), stop=(ec == EC - 1),
                       skip_group_check=True)
            if e2 == 3:
                dsl = dst[:, hp, 2 * sc2 * QCW:(2 * sc2 + 2) * QCW]
                nc.vector.tensor_copy(
                    dsl.rearrange("p (c q) -> p c q", c=2), ps)

        def emit_qk(hp, w_sb, dst, sc2):
            box = []
            for e2 in range(4):
                qk_quarter(hp, w_sb, dst, sc2, e2, box)

        def lazy_qk(hp, w_sb, dst, sc2):
            box = []
            for e2 in range(4):
                lazy.append((900, lambda a=e2: qk_quarter(
                    hp, w_sb, dst, sc2, a, box)))

        # serial prologue: K^T for head-pair 0 (all key positions) plus the
        # qc0/qc1 query slice of Q^T. V drains eagerly (one key-chunk-pair
        # per attention slot, always ahead of its attn@V consumer); the
        # remaining projections drain lazily (consumers 2+ iterations away).
        emit_qk(0, wk_sb, kt_sb, 0)
        emit_qk(0, wq_sb, qt_sb, 0)
        kt1_box = []
        for e2 in range(4):
            pend.append((900, lambda a=e2: qk_quarter(
                0, wk_sb, kt_sb, 1, a, kt1_box)))
        for sc2 in range(KC // 2):
            box = []
            eager.append(lambda s=sc2, b=box: v_half(s, 0, b))
            eager.append(lambda s=sc2, b=box: v_half(s, 1, b))
        lazy_qk(0, wq_sb, qt_sb, 1)
        for sc2 in range(NQC // 2):
            lazy_qk(1, wk_sb, kt_sb, sc2)
        for sc2 in range(NQC // 2):
            lazy_qk(1, wq_sb, qt_sb, sc2)

        # ---- phase 2: attention (64x64 array mode), deferred tails ----
        otn_tiles = {}

        def make_tail(hp, qc, exp_t, psO):
            def evac():
                ot = smallp.tile([P, QCW], F32, tag="ot")
                nc.vector.tensor_copy(ot, psO[:, 0, :])
                otf = smallp.tile([P, QCW], F32, tag="otf")
                nc.vector.tensor_add(otf, psO[:, 1, :], ot)
                otn_tiles[(hp, qc)] = (otf, None)

            def ones_blk(j0):
                s1 = otn_tiles[(hp, qc, "s1")]
                if j0 == 0:
                    otn_tiles[(hp, qc, "psS")] = psum.tile(
                        [P, 2, QCW], F32, tag="ps", name="psS")
                psS = otn_tiles[(hp, qc, "psS")]
                for h in range(2):
                    for g in range(2):
                        rg = slice(g * 64, (g + 1) * 64)
                        for j in range(j0, j0 + 4):
                            dst = (psS[h * 64:(h + 1) * 64, 0, :] if G_ACCUM
                                   else psS[h * 64:(h + 1) * 64, g, :])
                            mm(dst,
                               lhsT=ones[rg, :],
                               rhs=s1[rg, h, j, :],
                               start=(j == 0 and (g == 0 or not G_ACCUM)),
                               stop=(j == 7 and (g == 1 or not G_ACCUM)),
                               tile_position=(g * 64, h * 64),
                               skip_group_check=True)

            def norm():
                psS = otn_tiles[(hp, qc, "psS")]
                rec = smallp.tile([P, QCW], F32, tag="rec")
                if G_ACCUM:
                    nc.vector.reciprocal_approx_fast(rec, psS[:, 0, :])
                    otn = otnp.tile([P, QCW], BF16)
                    nc.vector.tensor_mul(otn, psO[:, 0, :], rec)
                else:
                    st = smallp.tile([P, QCW], F32, tag="st")
                    nc.vector.tensor_copy(st, psS[:, 0, :])
                    ssum = smallp.tile([P, QCW], F32, tag="ssum")
                    nc.vector.tensor_add(ssum, psS[:, 1, :], st)
                    nc.vector.reciprocal_approx_fast(rec, ssum)
                    otf, _ = otn_tiles[(hp, qc)]
                    otn = otnp.tile([P, QCW], BF16)
                    nc.vector.tensor_mul(otn, otf, rec)
                otn_tiles[(hp, qc)] = otn

            tail = [] if G_ACCUM else [(200, evac)]
            return tail + [(900, lambda: ones_blk(0)),
                           (900, lambda: ones_blk(4)), (400, norm)]

        def make_outproj(qc):
            qsl = slice(qc * QCW, (qc + 1) * QCW)

            def blk(m2):
                psP = psum.tile([P, 2, QCW], F32, tag="ps", name="psP")
                for i in range(2):
                    m = 2 * m2 + i
                    for hp2 in range(NHP):
                        mm(psP[:, i, :],
                           lhsT=wo_sb[:, hp2, m * P:(m + 1) * P],
                           rhs=otn_tiles[(hp2, qc)],
                           start=(hp2 == 0), stop=(hp2 == NHP - 1),
                           skip_group_check=True)
                outf = outfp.tile([P, 2, QCW], BF16)
                nc.vector.tensor_copy(outf, psP)
                (nc.sync if m2 % 2 == 0 else nc.gpsimd).dma_start(
                    outT[2 * m2 * P:(2 * m2 + 2) * P, qsl].rearrange(
                        "(c p) q -> p c q", p=P),
                    outf)

            return [(1900, lambda m=m2: blk(m)) for m2 in range(EC // 2)]

        for hp in range(NHP):
            for qc in range(NQC):
                qsl = slice(qc * QCW, (qc + 1) * QCW)
                exp_t = expp.tile([P, 2, KC, QCW], BF16)
                psO = psum_o.tile([P, 1 if G_ACCUM else 2, QCW], F32)
                for kcs in range(KC + 1):
                    if kcs < KC:
                        c = kcs
                        psL = psum.tile([P, 2, QCW], F32, tag="ps", name="psL")
                        for h in range(2):      # head of the pair
                            hg = slice(h * 64, (h + 1) * 64)
                            for p2 in range(2):  # key sub-chunk of 64
                                mm(psL[p2 * 64:(p2 + 1) * 64, h, :],
                                   lhsT=kt_sb[hg, hp, c * P + p2 * 64:c * P + (p2 + 1) * 64],
                                   rhs=qt_sb[hg, hp, qsl],
                                   start=True, stop=True,
                                   tile_position=(h * 64, p2 * 64),
                                   skip_group_check=True)
                        if c in SCHRAU_KC:
                            nc.vector.tensor_scalar(
                                exp_t[:, :, c, :].bitcast(mybir.dt.int16),
                                psL, S16 * SCALE, SCHRAU_BIAS,
                                mybir.AluOpType.mult, mybir.AluOpType.add)
                        else:
                            nc.scalar.activation(
                                exp_t[:, :, c, :], psL, EXP, scale=SCALE)
                    drain(1200, 500)
                    if kcs >= 1:
                        c = kcs - 1
                        for h in range(2):
                            col = hp * P + h * 64
                            for g in range(2):  # key sub-chunk = row group
                                rg = slice(g * 64, (g + 1) * 64)
                                dst = (psO[h * 64:(h + 1) * 64, 0, :] if G_ACCUM
                                       else psO[h * 64:(h + 1) * 64, g, :])
                                mm(dst,
                                   lhsT=v_sb[rg, c, col:col + 64],
                                   rhs=exp_t[rg, h, c, :],
                                   start=(c == 0 and (g == 0 or not G_ACCUM)),
                                   stop=(c == KC - 1 and (g == 1 or not G_ACCUM)),
                                   tile_position=(g * 64, h * 64),
                                   skip_group_check=True)
                    if kcs >= 9:
                        j = kcs - 9
                        if j == 0:
                            otn_tiles[(hp, qc, "s1")] = s1p.tile(
                                [P, 2, 8, QCW], BF16, name="s1", tag="s1")
                        s1 = otn_tiles[(hp, qc, "s1")]
                        # SBUF-only adds can run on the otherwise idle GpSimd
                        (nc.gpsimd if j < S1_GP_N else nc.vector).tensor_add(
                            s1[:, :, j, :], exp_t[:, :, j, :],
                            exp_t[:, :, j + 8, :])
                pend.extend(make_tail(hp, qc, exp_t, psO))
                if hp == 1:
                    lazy.extend(make_outproj(qc))
        drain(10**9, 10**9)


def _build():
    nc = bacc.Bacc("TRN2", debug=False, target_bir_lowering=False)
    with tile.TileContext(nc) as tc:
        _emit(tc)
    nc.compile()
    return nc


def _get_nc():
    global _NC
    if _NC is None:
        _NC = _build()
    return _NC


def make_in_maps(x, Wq, Wk, Wv, Wo):
    bf = ml_dtypes.bfloat16
    x = np.asarray(x, np.float32)
    xTb = [np.ascontiguousarray(x[b].T).astype(bf) for b in range(B)]
    WqT = np.ascontiguousarray(np.asarray(Wq, np.float32).T).astype(bf)
    WkT = np.ascontiguousarray(np.asarray(Wk, np.float32).T).astype(bf)
    WvT = np.ascontiguousarray(np.asarray(Wv, np.float32).T).astype(bf)
    WoT = np.ascontiguousarray(np.asarray(Wo, np.float32).T).astype(bf)

    in_maps = []
    for c in range(NCORES):
        b, hg = divmod(c, GROUPS)
        sl = slice(hg * DHG, (hg + 1) * DHG)
        in_maps.append({
            "xT": xTb[b],
            "wqT": np.ascontiguousarray(WqT[:, sl]),
            "wkT": np.ascontiguousarray(WkT[:, sl]),
            "wvT": np.ascontiguousarray(WvT[:, sl]),
            "woT": np.ascontiguousarray(WoT[sl, :]),
        })
    return in_maps


def run(in_maps, **kwargs):
    nc = _get_nc()
    return bass_utils.run_bass_kernel_spmd(
        nc, in_maps, core_ids=list(range(NCORES)), **kwargs)


def assemble(outs, bo):
    bo = np.asarray(bo, np.float32)
    out = np.empty((B, S, E), np.float32)
    for b in range(B):
        acc = outs[b * GROUPS]["outT"].astype(np.float32)
        for hg in range(1, GROUPS):
            acc += outs[b * GROUPS + hg]["outT"].astype(np.float32)
        out[b] = acc.T + bo
    return out


def kernel(x, Wq, Wk, Wv, Wo, bo):
    in_maps = make_in_maps(x, Wq, Wk, Wv, Wo)
    res = run(in_maps)
    return assemble(res.results, bo)


# revision 37
# speedup vs baseline: 1.1487x; 1.1487x over previous
"""Multi-head attention (B=2, S=2048, E=1024, H=16, D=64) on 8 TRN2 NeuronCores.

Sharding: data parallel over batch (2) x head-group parallel (4 groups of 4
heads). Each core computes Q/K/V projections for its 4 heads, full-sequence
attention for those heads, and a partial output projection (its heads' rows of
Wo). The host sums the 4 partial outputs per batch and adds the bias.

On-device layout (all matmul operands bf16, accumulation fp32):
  - x is fed pre-transposed per batch: xT [E, S]. Input DMA is ordered
    first-needed-first: x in 512-wide s-slices in the (ec-pair, s-half) order
    the K0 prologue consumes them, round-robin across three DMA queues;
    weights go chunk-granular on the scalar queue so the first matmul waits
    only for its own wk pair + 4 x chunks (~4us) instead of the full loads.
  - Q^T, K^T are computed head-transposed [dh, S] so that attention logits are
    produced key-partitioned: logitsT[k, q] = sum_d K^T[d,k] Q^T[d,q]. Softmax
    (no max subtraction -- logits are provably tiny) is exp on ScalarE plus a
    partition-reduction done with an all-ones matmul; attn@V then consumes the
    exp tiles directly as the moving operand with V [s, dh] as stationary.
  - attn@V and the ones-reduction accumulate both 64-row key-subchunks into
    the SAME psum region (the psum accumulate port makes concurrent quadrant
    drains safe), so psO/psS are one bank each: psO double-buffers (removing
    the per-iteration WAR stall) and the tail is rec+mul straight out of PSUM.
  - Projections and the output projection run the PE in plain 128x128 mode;
    the attention core (logits / attn@V / ones-reduction) runs in 64x64 array
    tiling so the d=64-contraction matmuls use the full array.
  - Q/K projections run ec-outer/i-inner so consecutive matmuls share their
    stationary weight chunk; the second K block drains via the urgent queue
    inside the first attention slots instead of serializing the prologue.
  - PSUM evacuations for V and the output projection run on GpSimd (idle
    otherwise); VectorE keeps the exp-tree adds and the softmax tail so the
    ScalarE exp cadence and the PE stay the pacers. Output is stored bf16 and
    summed on host.
"""

import os
import numpy as np
import ml_dtypes

import concourse.bass as bass
import concourse.mybir as mybir
import concourse.tile as tile
from concourse import bacc
from concourse import bass_utils
from contextlib import ExitStack

# bisect flags (default = all optimizations on)
# 0 = baseline DMA scheme. The restructured schemes (1/2) measured neutral
# (prologue is bounded by the ~8.7us DMA-ring spin-up, not trigger order),
# and scheme 2 correlated with a chip clock-state that runs all non-PE
# engines at 1.0GHz instead of 1.2GHz (+19% total time).
DMA_V2 = int(os.environ.get("K_DMA_V2", "0"))
# same-region concurrent-quadrant PSUM accumulation fails at runtime on hw
# (works in CoreSim) -- keep disjoint g-regions + evac add.
G_ACCUM = os.environ.get("K_G_ACCUM", "0") == "1"
# number of the 8 per-iteration s1 tree-adds to run on GpSimd (2.1us each
# there vs 0.4us on DVE -- GpSimd saturates above ~4)
S1_GP_N = int(os.environ.get("K_S1_GP_N", "0"))
# kc slots whose exp runs as a one-op Schraudolph approximation on the DVE
# (int16 bitcast trick) instead of ScalarE: relieves the exp cadence.
# Max rel err of the approx ~3.3%; softmax normalization cancels most of it
# (measured end-to-end: 3 slots -> 0.0061 vs tolerance 2e-2).
SCHRAU_KC = frozenset(
    int(c) for c in os.environ.get("K_SCHRAU_KC", "").split(",") if c != "")
S16 = 128.0 / float(np.log(2.0))
SCHRAU_BIAS = 16256.0 - 5.0  # truncation-calibrated sigma
# double-buffer psO (psum shared pool drops to 2 bufs to fit 8 banks).
# Measured: 3-deep psL rotation matters more -- keep 0.
PSO2 = os.environ.get("K_PSO2", "0") == "1"
# deferred-work pacing (ns of estimated PE work per kc slot)
B_URGENT = int(os.environ.get("K_B_URGENT", "1200"))
B_LAZY = int(os.environ.get("K_B_LAZY", "420"))
OP_COST = int(os.environ.get("K_OP_COST", "2000"))
# second-level exp-tree adds (s2 = s1[j]+s1[j+4]) on GpSimd: halves the
# ones-matmul PE work at the cost of 4 slow (2.1us) adds on idle GpSimd
S2_GP = os.environ.get("K_S2_GP", "0") == "1"

P = 128
B, S, E = 2, 2048, 1024
H, D = 16, 64
NCORES = 8
GROUPS = NCORES // B          # 4 head-groups per batch
HPG = H // GROUPS             # 4 heads per core
DHG = HPG * D                 # 256 head dims per core
NHP = HPG // 2                # 2 head-pairs per core
EC = E // P                   # 8 e-chunks of 128
KC = S // P                   # 16 key chunks of 128
QCW = 512                     # q-chunk width
NQC = S // QCW                # 4 q chunks
SCALE = float(D) ** -0.5

BF16 = mybir.dt.bfloat16
F32 = mybir.dt.float32
EXP = mybir.ActivationFunctionType.Exp

_NC = None


def _emit(tc):
    nc = tc.nc
    xT = nc.dram_tensor("xT", [E, S], BF16, kind="ExternalInput").ap()
    wqT = nc.dram_tensor("wqT", [E, DHG], BF16, kind="ExternalInput").ap()
    wkT = nc.dram_tensor("wkT", [E, DHG], BF16, kind="ExternalInput").ap()
    wvT = nc.dram_tensor("wvT", [E, DHG], BF16, kind="ExternalInput").ap()
    woT = nc.dram_tensor("woT", [DHG, E], BF16, kind="ExternalInput").ap()
    outT = nc.dram_tensor("outT", [E, S], BF16, kind="ExternalOutput").ap()

    mm = nc.tensor.matmul

    with ExitStack() as ctx:
        consts = ctx.enter_context(tc.tile_pool(name="consts", bufs=1))
        xp = ctx.enter_context(tc.tile_pool(name="xp", bufs=1))
        qkvp = ctx.enter_context(tc.tile_pool(name="qkvp", bufs=1))
        psum = ctx.enter_context(tc.tile_pool(
            name="psum", bufs=2 if PSO2 else 3, space="PSUM"))
        psum_o = ctx.enter_context(tc.tile_pool(
            name="psum_o", bufs=2 if (G_ACCUM or PSO2) else 1, space="PSUM"))
        expp = ctx.enter_context(tc.tile_pool(name="expp", bufs=2))
        s1p = ctx.enter_context(tc.tile_pool(name="s1p", bufs=1))
        s2p = ctx.enter_context(tc.tile_pool(name="s2p", bufs=1))
        smallp = ctx.enter_context(tc.tile_pool(name="smallp", bufs=2))
        otnp = ctx.enter_context(tc.tile_pool(name="otnp", bufs=6))
        outfp = ctx.enter_context(tc.tile_pool(name="outfp", bufs=3))

        # ---- input loads: x first-needed-first on 3 queues, weights
        # chunk-granular on the (initially idle) scalar queue so the first
        # matmul waits only for wk pair 0 and the first 4 x chunks.
        wq_sb = consts.tile([P, EC, DHG], BF16)
        wk_sb = consts.tile([P, EC, DHG], BF16)
        wv_sb = consts.tile([P, EC, DHG], BF16)
        wo_sb = consts.tile([P, NHP, E], BF16)
        ones = consts.tile([P, 64], BF16)
        nc.vector.memset(ones, 1.0)
        x_sb = xp.tile([P, EC, S], BF16)
        dma_eng = [nc.sync, nc.scalar, nc.gpsimd]
        if DMA_V2 == 2:
            # chunked weights on scalar, ec-pair-merged x on sync+gpsimd
            wkr = wkT.rearrange("(c p) d -> p c d", p=P)
            wqr = wqT.rearrange("(c p) d -> p c d", p=P)
            xr = xT.rearrange("(c p) q -> p c q", p=P)
            for e2 in range(4):
                nc.scalar.dma_start(wk_sb[:, 2 * e2:2 * e2 + 2, :],
                                    wkr[:, 2 * e2:2 * e2 + 2, :])
            for e2 in range(4):
                nc.scalar.dma_start(wq_sb[:, 2 * e2:2 * e2 + 2, :],
                                    wqr[:, 2 * e2:2 * e2 + 2, :])
            nc.scalar.dma_start(wv_sb, wvT.rearrange("(c p) d -> p c d", p=P))
            nc.scalar.dma_start(wo_sb, woT.rearrange("(h p) e -> p h e", p=P))
            order = [(e2, sq) for e2 in range(4) for sq in (0, 1)]
            order += [(e2, sq) for sq in (2, 3) for e2 in range(4)]
            for n, (e2, sq) in enumerate(order):
                ssl = slice(sq * QCW, (sq + 1) * QCW)
                (nc.sync if n % 2 == 0 else nc.gpsimd).dma_start(
                    x_sb[:, 2 * e2:2 * e2 + 2, ssl],
                    xr[:, 2 * e2:2 * e2 + 2, ssl])
        elif DMA_V2 == 1:
            # baseline scheme + only wk split in half and x ec-pair order,
            # to pull the first matmul earlier without changing queue shape
            wkr = wkT.rearrange("(c p) d -> p c d", p=P)
            nc.scalar.dma_start(wk_sb[:, 0:2, :], wkr[:, 0:2, :])
            nc.scalar.dma_start(wk_sb[:, 2:EC, :], wkr[:, 2:EC, :])
            order = [(ec, sq) for e2 in range(4) for ec in (2 * e2, 2 * e2 + 1)
                     for sq in (0, 1)]
            order += [(ec, sq) for sq in (2, 3) for ec in range(EC)]
            for n, (ec, sq) in enumerate(order):
                ssl = slice(sq * QCW, (sq + 1) * QCW)
                dma_eng[n % 3].dma_start(
                    x_sb[:, ec, ssl], xT[ec * P:(ec + 1) * P, ssl])
                if n == 15:
                    nc.scalar.dma_start(
                        wq_sb, wqT.rearrange("(c p) d -> p c d", p=P))
                if n == 23:
                    nc.scalar.dma_start(
                        wv_sb, wvT.rearrange("(c p) d -> p c d", p=P))
            nc.scalar.dma_start(wo_sb, woT.rearrange("(h p) e -> p h e", p=P))
        else:
            # baseline scheme; DMA_V2==3 only pulls the wq/wv triggers to
            # earlier positions on the scalar queue (same queue shapes).
            wq_pos, wv_pos = (6, 12) if DMA_V2 == 3 else (15, 23)
            nc.scalar.dma_start(wk_sb, wkT.rearrange("(c p) d -> p c d", p=P))
            order = [(ec, sq) for ec in range(EC) for sq in (0, 1)]
            order += [(ec, sq) for sq in (2, 3) for ec in range(EC)]
            for n, (ec, sq) in enumerate(order):
                ssl = slice(sq * QCW, (sq + 1) * QCW)
                dma_eng[n % 3].dma_start(
                    x_sb[:, ec, ssl], xT[ec * P:(ec + 1) * P, ssl])
                if n == wq_pos:
                    nc.scalar.dma_start(
                        wq_sb, wqT.rearrange("(c p) d -> p c d", p=P))
                if n == wv_pos:
                    nc.scalar.dma_start(
                        wv_sb, wvT.rearrange("(c p) d -> p c d", p=P))
            nc.scalar.dma_start(wo_sb, woT.rearrange("(h p) e -> p h e", p=P))

        qt_sb = qkvp.tile([P, NHP, S], BF16)   # [dh-pair, hp, s]
        kt_sb = qkvp.tile([P, NHP, S], BF16)
        v_sb = qkvp.tile([P, KC, DHG], BF16)   # [s%128, s-chunk, dh]

        # deferred-work queue: (approx_pe_ns, closure) drained into kc slots
        eager = []  # V chunks: exactly one per slot, always ahead of attnV
        pend = []   # urgent: per-iteration tails (free pools quickly)
        lazy = []   # deferred projections + output projection
        acc = [0.0, 0.0]

        def drain(b_urgent, b_lazy):
            acc[0] += b_urgent
            while pend and pend[0][0] <= acc[0]:
                cost, fn = pend.pop(0)
                fn()
                acc[0] -= cost
            if not pend:
                acc[0] = 0.0
            if eager:
                eager.pop(0)()
                return
            acc[1] += b_lazy
            while lazy and lazy[0][0] <= acc[1]:
                cost, fn = lazy.pop(0)
                fn()
                acc[1] -= cost
            if not lazy:
                acc[1] = 0.0

        # ---- phase 1 emitters (plain 128x128 array mode) ----
        def v_half(sc2, i, box):
            if i == 0:
                box.append(psum.tile([P, 2, QCW], F32, tag="ps", name="psV"))
            ps = box[0]
            sc = 2 * sc2 + i
            for ec in range(EC):
                mm(ps[:, i, 0:DHG],
                   lhsT=x_sb[:, ec, sc * P:(sc + 1) * P],
                   rhs=wv_sb[:, ec, :],
                   start=(ec == 0), stop=(ec == EC - 1),
                   skip_group_check=True)
            nc.vector.tensor_copy(v_sb[:, sc:sc + 1, :], ps[:, i:i + 1, 0:DHG])

        def qk_quarter(hp, w_sb, dst, sc2, e2, box):
            # ec-outer / i-inner: the two 512-q halves share each weight
            # chunk, so consecutive matmuls keep the same stationary.
            if e2 == 0:
                box.append(psum.tile([P, 2, QCW], F32, tag="ps", name="psQK"))
            ps = box[0]
            for ec in range(e2 * 2, e2 * 2 + 2):
                for i in range(2):
                    ssl = slice((2 * sc2 + i) * QCW, (2 * sc2 + i + 1) * QCW)
                    mm(ps[:, i, :],
                       lhsT=w_sb[:, ec, hp * P:(hp + 1) * P],
                       rhs=x_sb[:, ec, ssl],
                       start=(ec == 0), stop=(ec == EC - 1),
                       skip_group_check=True)
            if e2 == 3:
                dsl = dst[:, hp, 2 * sc2 * QCW:(2 * sc2 + 2) * QCW]
                nc.vector.tensor_copy(
                    dsl.rearrange("p (c q) -> p c q", c=2), ps)

        def emit_qk(hp, w_sb, dst, sc2):
            box = []
            for e2 in range(4):
                qk_quarter(hp, w_sb, dst, sc2, e2, box)

        def lazy_qk(hp, w_sb, dst, sc2):
            box = []
            for e2 in range(4):
                lazy.append((900, lambda a=e2: qk_quarter(
                    hp, w_sb, dst, sc2, a, box)))

        # serial prologue: K^T for head-pair 0 (all key positions) plus the
        # qc0/qc1 query slice of Q^T. V drains eagerly (one key-chunk-pair
        # per attention slot, always ahead of its attn@V consumer); the
        # remaining projections drain lazily (consumers 2+ iterations away).
        emit_qk(0, wk_sb, kt_sb, 0)
        emit_qk(0, wq_sb, qt_sb, 0)
        kt1_box = []
        for e2 in range(4):
            pend.append((900, lambda a=e2: qk_quarter(
                0, wk_sb, kt_sb, 1, a, kt1_box)))
        for sc2 in range(KC // 2):
            box = []
            eager.append(lambda s=sc2, b=box: v_half(s, 0, b))
            eager.append(lambda s=sc2, b=box: v_half(s, 1, b))
        lazy_qk(0, wq_sb, qt_sb, 1)
        for sc2 in range(NQC // 2):
            lazy_qk(1, wk_sb, kt_sb, sc2)
        for sc2 in range(NQC // 2):
            lazy_qk(1, wq_sb, qt_sb, sc2)

        # ---- phase 2: attention (64x64 array mode), deferred tails ----
        otn_tiles = {}

        def make_tail(hp, qc, exp_t, psO):
            def evac():
                ot = smallp.tile([P, QCW], F32, tag="ot")
                nc.vector.tensor_copy(ot, psO[:, 0, :])
                otf = smallp.tile([P, QCW], F32, tag="otf")
                nc.vector.tensor_add(otf, psO[:, 1, :], ot)
                otn_tiles[(hp, qc)] = (otf, None)

            def ones_blk(j0):
                src = otn_tiles[(hp, qc, "s2" if S2_GP else "s1")]
                jhi = 3 if S2_GP else 7
                if j0 == 0:
                    otn_tiles[(hp, qc, "psS")] = psum.tile(
                        [P, 2, QCW], F32, tag="ps", name="psS")
                psS = otn_tiles[(hp, qc, "psS")]
                for h in range(2):
                    for g in range(2):
                        rg = slice(g * 64, (g + 1) * 64)
                        for j in range(j0, j0 + 4):
                            dst = (psS[h * 64:(h + 1) * 64, 0, :] if G_ACCUM
                                   else psS[h * 64:(h + 1) * 64, g, :])
                            mm(dst,
                               lhsT=ones[rg, :],
                               rhs=src[rg, h, j, :],
                               start=(j == 0 and (g == 0 or not G_ACCUM)),
                               stop=(j == jhi and (g == 1 or not G_ACCUM)),
                               tile_position=(g * 64, h * 64),
                               skip_group_check=True)

            def norm():
                psS = otn_tiles[(hp, qc, "psS")]
                rec = smallp.tile([P, QCW], F32, tag="rec")
                if G_ACCUM:
                    nc.vector.reciprocal_approx_fast(rec, psS[:, 0, :])
                    otn = otnp.tile([P, QCW], BF16)
                    nc.vector.tensor_mul(otn, psO[:, 0, :], rec)
                else:
                    st = smallp.tile([P, QCW], F32, tag="st")
                    nc.vector.tensor_copy(st, psS[:, 0, :])
                    ssum = smallp.tile([P, QCW], F32, tag="ssum")
                    nc.vector.tensor_add(ssum, psS[:, 1, :], st)
                    nc.vector.reciprocal_approx_fast(rec, ssum)
                    otf, _ = otn_tiles[(hp, qc)]
                    otn = otnp.tile([P, QCW], BF16)
                    nc.vector.tensor_mul(otn, otf, rec)
                otn_tiles[(hp, qc)] = otn

            tail = [] if G_ACCUM else [(200, evac)]
            if S2_GP:
                return tail + [(900, lambda: ones_blk(0)), (400, norm)]
            return tail + [(900, lambda: ones_blk(0)),
                           (900, lambda: ones_blk(4)), (400, norm)]

        def make_outproj(qc):
            qsl = slice(qc * QCW, (qc + 1) * QCW)

            def blk(m2):
                psP = psum.tile([P, 2, QCW], F32, tag="ps", name="psP")
                for i in range(2):
                    m = 2 * m2 + i
                    for hp2 in range(NHP):
                        mm(psP[:, i, :],
                           lhsT=wo_sb[:, hp2, m * P:(m + 1) * P],
                           rhs=otn_tiles[(hp2, qc)],
                           start=(hp2 == 0), stop=(hp2 == NHP - 1),
                           skip_group_check=True)
                outf = outfp.tile([P, 2, QCW], BF16)
                nc.vector.tensor_copy(outf, psP)
                (nc.sync if m2 % 2 == 0 else nc.gpsimd).dma_start(
                    outT[2 * m2 * P:(2 * m2 + 2) * P, qsl].rearrange(
                        "(c p) q -> p c q", p=P),
                    outf)

            return [(OP_COST, lambda m=m2: blk(m)) for m2 in range(EC // 2)]

        for hp in range(NHP):
            for qc in range(NQC):
                qsl = slice(qc * QCW, (qc + 1) * QCW)
                exp_t = expp.tile([P, 2, KC, QCW], BF16)
                psO = psum_o.tile([P, 1 if G_ACCUM else 2, QCW], F32)
                for kcs in range(KC + 1):
                    if kcs < KC:
                        c = kcs
                        psL = psum.tile([P, 2, QCW], F32, tag="ps", name="psL")
                        for h in range(2):      # head of the pair
                            hg = slice(h * 64, (h + 1) * 64)
                            for p2 in range(2):  # key sub-chunk of 64
                                mm(psL[p2 * 64:(p2 + 1) * 64, h, :],
                                   lhsT=kt_sb[hg, hp, c * P + p2 * 64:c * P + (p2 + 1) * 64],
                                   rhs=qt_sb[hg, hp, qsl],
                                   start=True, stop=True,
                                   tile_position=(h * 64, p2 * 64),
                                   skip_group_check=True)
                        if c in SCHRAU_KC:
                            nc.vector.tensor_scalar(
                                exp_t[:, :, c, :].bitcast(mybir.dt.int16),
                                psL, S16 * SCALE, SCHRAU_BIAS,
                                mybir.AluOpType.mult, mybir.AluOpType.add)
                        else:
                            nc.scalar.activation(
                                exp_t[:, :, c, :], psL, EXP, scale=SCALE)
                    drain(B_URGENT, B_LAZY)
                    if kcs >= 1:
                        c = kcs - 1
                        for h in range(2):
                            col = hp * P + h * 64
                            for g in range(2):  # key sub-chunk = row group
                                rg = slice(g * 64, (g + 1) * 64)
                                dst = (psO[h * 64:(h + 1) * 64, 0, :] if G_ACCUM
                                       else psO[h * 64:(h + 1) * 64, g, :])
                                mm(dst,
                                   lhsT=v_sb[rg, c, col:col + 64],
                                   rhs=exp_t[rg, h, c, :],
                                   start=(c == 0 and (g == 0 or not G_ACCUM)),
                                   stop=(c == KC - 1 and (g == 1 or not G_ACCUM)),
                                   tile_position=(g * 64, h * 64),
                                   skip_group_check=True)
                    if kcs >= 9:
                        j = kcs - 9
                        if j == 0:
                            otn_tiles[(hp, qc, "s1")] = s1p.tile(
                                [P, 2, 8, QCW], BF16, name="s1", tag="s1")
                        s1 = otn_tiles[(hp, qc, "s1")]
                        # SBUF-only adds can run on the otherwise idle GpSimd
                        (nc.gpsimd if j < S1_GP_N else nc.vector).tensor_add(
                            s1[:, :, j, :], exp_t[:, :, j, :],
                            exp_t[:, :, j + 8, :])
                    if S2_GP and kcs >= 13:
                        j2 = kcs - 13
                        if j2 == 0:
                            otn_tiles[(hp, qc, "s2")] = s2p.tile(
                                [P, 2, 4, QCW], BF16, name="s2", tag="s2")
                        s2 = otn_tiles[(hp, qc, "s2")]
                        s1 = otn_tiles[(hp, qc, "s1")]
                        nc.gpsimd.tensor_add(
                            s2[:, :, j2, :], s1[:, :, j2, :],
                            s1[:, :, j2 + 4, :])
                pend.extend(make_tail(hp, qc, exp_t, psO))
                if hp == 1:
                    lazy.extend(make_outproj(qc))
        drain(10**9, 10**9)


def _build():
    nc = bacc.Bacc("TRN2", debug=False, target_bir_lowering=False)
    with tile.TileContext(nc) as tc:
        _emit(tc)
    nc.compile()
    return nc


def _get_nc():
    global _NC
    if _NC is None:
        _NC = _build()
    return _NC


def make_in_maps(x, Wq, Wk, Wv, Wo):
    bf = ml_dtypes.bfloat16
    x = np.asarray(x, np.float32)
    xTb = [np.ascontiguousarray(x[b].T).astype(bf) for b in range(B)]
    WqT = np.ascontiguousarray(np.asarray(Wq, np.float32).T).astype(bf)
    WkT = np.ascontiguousarray(np.asarray(Wk, np.float32).T).astype(bf)
    WvT = np.ascontiguousarray(np.asarray(Wv, np.float32).T).astype(bf)
    WoT = np.ascontiguousarray(np.asarray(Wo, np.float32).T).astype(bf)

    in_maps = []
    for c in range(NCORES):
        b, hg = divmod(c, GROUPS)
        sl = slice(hg * DHG, (hg + 1) * DHG)
        in_maps.append({
            "xT": xTb[b],
            "wqT": np.ascontiguousarray(WqT[:, sl]),
            "wkT": np.ascontiguousarray(WkT[:, sl]),
            "wvT": np.ascontiguousarray(WvT[:, sl]),
            "woT": np.ascontiguousarray(WoT[sl, :]),
        })
    return in_maps


def run(in_maps, **kwargs):
    nc = _get_nc()
    return bass_utils.run_bass_kernel_spmd(
        nc, in_maps, core_ids=list(range(NCORES)), **kwargs)


def assemble(outs, bo):
    bo = np.asarray(bo, np.float32)
    out = np.empty((B, S, E), np.float32)
    for b in range(B):
        acc = outs[b * GROUPS]["outT"].astype(np.float32)
        for hg in range(1, GROUPS):
            acc += outs[b * GROUPS + hg]["outT"].astype(np.float32)
        out[b] = acc.T + bo
    return out


def kernel(x, Wq, Wk, Wv, Wo, bo):
    in_maps = make_in_maps(x, Wq, Wk, Wv, Wo)
    res = run(in_maps)
    return assemble(res.results, bo)
